# revision 1
# baseline (speedup 1.0000x reference)
"""OHEM cross-entropy loss kernel for Trainium2 (8 NeuronCores, Bass/Tile).

Math (matches reference.py):
    logp   = log_softmax(seg_logit, axis=1)          # [B,C,H,W], C=19
    x_l    = logp at label (ignore 255 -> class 0)
    prob   = exp(x_l)
    thr    = max(sort(prob.flatten())[MIN_KEPT*B], 0.7)
    loss   = mean(-x_l * (prob < thr))

Device strategy (data-parallel over B across 8 cores, one image per core):
    For each pixel p:  t = x_raw[label] - log(sum_c exp(x_raw[c]))  (= logp at label)
    w = 1[t < log(0.7)]   (valid when count(prob<0.7) > MIN_KEPT*B, which the
                           host verifies from the returned counts; otherwise a
                           host fallback computes the exact quantile path)
    Per-core partial sums of (t - log .7)*w (via min(u,0)) and of w are
    returned as [128, 16] partials; host combines:
        sum(-t*w) = -(sum_min + log(.7)*count)

    On-chip per 128x512-pixel chunk:
      - one fat DMA loads [128, 19, 512] f32 logits (class-major)
      - ACT: 19x exp -> bf16 expbuf; pairwise bulk adds (DVE 2x bf16) -> sumexp
      - label gather: 18 in-place copy_predicated mux-tree merges keyed on
        host-provided label bit-planes -> x_l in slot 0
      - ACT: lse = Ln(sumexp); DVE: u = (x_l - log.7) - lse;
        tensor_scalar accum_out reductions of min(u,0) and 1[u<0]
"""

import numpy as np

B = 8
C = 19
H, W = 512, 1024
HW = H * W            # 524288 pixels per image/core
P = 128               # SBUF partitions
FREE = HW // P        # 4096 pixels per partition
F = 512               # chunk free size
NCHUNK = FREE // F    # 8
NBITS = 5             # ceil(log2(19))
C0 = float(np.log(np.float32(0.7)))
MIN_KEPT = 100000
IGNORE_INDEX = 255
N_TOTAL = B * HW

_CACHE = {}


def _build_nc():
    import concourse.bacc as bacc
    import concourse.mybir as mybir
    import concourse.tile as tile

    fp32 = mybir.dt.float32
    bf16 = mybir.dt.bfloat16
    u8 = mybir.dt.uint8

    # Bacc (not plain Bass): its compile pass splits multi-sem sync waits,
    # which the mux-tree copy_predicated instructions need.
    nc = bacc.Bacc()
    logit = nc.dram_tensor("logit", [C, HW], fp32, kind="ExternalInput")
    bits = nc.dram_tensor("bits", [NBITS, P, FREE], u8, kind="ExternalInput")
    acc = nc.dram_tensor("acc", [P, 2 * NCHUNK], fp32, kind="ExternalOutput")

    # [C, (P FREE)] -> [P, C, FREE] view for chunked class-major loads
    logit_v = logit[:, :].rearrange("c (p f) -> p c f", p=P)

    with tile.TileContext(nc) as tc:
        with (
            tc.tile_pool(name="lb", bufs=2) as lb_pool,
            tc.tile_pool(name="eb", bufs=1) as eb_pool,
            tc.tile_pool(name="bits", bufs=1) as bits_pool,
            tc.tile_pool(name="pix", bufs=2) as pix_pool,
            tc.tile_pool(name="accp", bufs=1) as acc_pool,
        ):
            acc_t = acc_pool.tile([P, 2 * NCHUNK], fp32)
            bits_t = bits_pool.tile([P, NBITS, FREE], u8)
            # all 5 bit-planes in one DMA: [NBITS, P, FREE] -> [P, NBITS, FREE]
            nc.sync.dma_start(
                out=bits_t[:], in_=bits[:, :, :].rearrange("k p f -> p k f")
            )

            for j in range(NCHUNK):
                lb = lb_pool.tile([P, C, F], fp32, tag="lb")
                nc.sync.dma_start(out=lb[:], in_=logit_v[:, :, j * F : (j + 1) * F])

                eb = eb_pool.tile([P, C, F], bf16, tag="eb")
                for c in range(C):
                    nc.scalar.activation(
                        out=eb[:, c, :],
                        in_=lb[:, c, :],
                        func=mybir.ActivationFunctionType.Exp,
                    )

                # sumexp: pairwise bulk adds in bf16 (2x mode), final add in f32
                # tree: [0:9]+=[9:18]; [0:4]+=[4:8]; [8]+=[18]; [0:2]+=[2:4];
                #       [0]+=[1]; sumexp = [0]+[8] (f32 out)
                nc.vector.tensor_tensor(
                    out=eb[:, 0:9, :], in0=eb[:, 0:9, :], in1=eb[:, 9:18, :],
                    op=mybir.AluOpType.add,
                )
                nc.vector.tensor_tensor(
                    out=eb[:, 0:4, :], in0=eb[:, 0:4, :], in1=eb[:, 4:8, :],
                    op=mybir.AluOpType.add,
                )
                nc.vector.tensor_tensor(
                    out=eb[:, 8, :], in0=eb[:, 8, :], in1=eb[:, 18, :],
                    op=mybir.AluOpType.add,
                )
                nc.vector.tensor_tensor(
                    out=eb[:, 0:2, :], in0=eb[:, 0:2, :], in1=eb[:, 2:4, :],
                    op=mybir.AluOpType.add,
                )
                nc.vector.tensor_tensor(
                    out=eb[:, 0, :], in0=eb[:, 0, :], in1=eb[:, 1, :],
                    op=mybir.AluOpType.add,
                )
                sumexp = pix_pool.tile([P, F], fp32, tag="sumexp")
                nc.vector.tensor_tensor(
                    out=sumexp[:], in0=eb[:, 0, :], in1=eb[:, 8, :],
                    op=mybir.AluOpType.add,
                )

                lse = pix_pool.tile([P, F], fp32, tag="lse")
                nc.scalar.activation(
                    out=lse[:], in_=sumexp[:], func=mybir.ActivationFunctionType.Ln
                )

                # label mux-tree gather, in place on lb; merge (a, b, bit):
                # lb[:,a,:] <- lb[:,b,:] where bit-plane set
                merges = [
                    *[(2 * i, 2 * i + 1, 0) for i in range(9)],     # bit 0
                    (0, 2, 1), (4, 6, 1), (8, 10, 1), (12, 14, 1), (16, 18, 1),
                    (0, 4, 2), (8, 12, 2),                          # bit 2
                    (0, 8, 3),                                      # bit 3
                    (0, 16, 4),                                     # bit 4
                ]
                bslice = bits_t[:, :, j * F : (j + 1) * F]
                for a, b, k in merges:
                    nc.vector.copy_predicated(
                        out=lb[:, a, :], mask=bslice[:, k, :], data=lb[:, b, :]
                    )

                # u = (x_l - log0.7) - lse; partials: sum(min(u,0)), count(u<0)
                u = pix_pool.tile([P, F], fp32, tag="u")
                nc.vector.scalar_tensor_tensor(
                    out=u[:], in0=lb[:, 0, :], scalar=C0, in1=lse[:],
                    op0=mybir.AluOpType.subtract, op1=mybir.AluOpType.subtract,
                )
                # with accum_out, op1 is the reduce op: accum = reduce(out, op1)
                scr = pix_pool.tile([P, F], fp32, tag="scr")
                nc.vector.tensor_scalar(
                    out=scr[:], in0=u[:], scalar1=0.0, scalar2=None,
                    op0=mybir.AluOpType.min, op1=mybir.AluOpType.add,
                    accum_out=acc_t[:, j : j + 1],
                )
                scr2 = pix_pool.tile([P, F], fp32, tag="scr2")
                nc.vector.tensor_scalar(
                    out=scr2[:], in0=u[:], scalar1=0.0, scalar2=None,
                    op0=mybir.AluOpType.is_lt, op1=mybir.AluOpType.add,
                    accum_out=acc_t[:, NCHUNK + j : NCHUNK + j + 1],
                )

            nc.sync.dma_start(out=acc[:, :], in_=acc_t[:])
    nc.finalize()  # Bacc: runs compile() (reg alloc, sync-wait splitting)
    return nc


def _host_fallback(seg_logit, seg_label):
    """Exact numpy replication of the reference (quantile path included)."""
    x = np.asarray(seg_logit, dtype=np.float32)
    lbl = np.asarray(seg_label)
    Bn, Cn = x.shape[0], x.shape[1]
    xf = x.reshape(Bn, Cn, -1)
    m = xf.max(axis=1, keepdims=True)
    e = np.exp(xf - m)
    lse = np.log(e.sum(axis=1, keepdims=True)) + m
    logp = xf - lse
    l2 = np.where(lbl == IGNORE_INDEX, 0, lbl).reshape(Bn, 1, -1).astype(np.int64)
    lp_at = np.take_along_axis(logp, l2, axis=1)[:, 0]
    prob = np.exp(lp_at)
    sortp = np.sort(prob.reshape(-1))
    idx = min(MIN_KEPT * Bn, sortp.shape[0] - 1)
    thr = max(float(sortp[idx]), np.float32(0.7))
    wgt = (prob < thr).astype(np.float32)
    return np.float32((-lp_at * wgt).mean())


def kernel(seg_logit, seg_label):
    from concourse import bass_utils

    x = np.ascontiguousarray(np.asarray(seg_logit, dtype=np.float32)).reshape(
        B, C, HW
    )
    lbl = np.asarray(seg_label)
    lbl = np.where(lbl == IGNORE_INDEX, 0, lbl).astype(np.uint8).reshape(B, P, FREE)
    # 5 bit-planes per core: [NBITS, P, FREE] uint8
    bits = np.stack(
        [((lbl >> k) & 1).astype(np.uint8) for k in range(NBITS)], axis=1
    )  # [B, NBITS, P, FREE]

    if "nc" not in _CACHE:
        _CACHE["nc"] = _build_nc()
    nc = _CACHE["nc"]

    in_maps = [{"logit": x[b], "bits": bits[b]} for b in range(B)]
    res = bass_utils.run_bass_kernel_spmd(nc, in_maps, core_ids=list(range(B)))

    racc = 0.0
    wacc = 0.0
    for r in res.results:
        a = r["acc"]
        racc += float(a[:, :NCHUNK].sum(dtype=np.float64))
        wacc += float(a[:, NCHUNK:].sum(dtype=np.float64))

    if wacc <= MIN_KEPT * B:
        # quantile threshold exceeds 0.7 -> exact host path (rare/never for
        # the target distribution)
        return _host_fallback(seg_logit, seg_label)

    total = -(racc + C0 * wacc)
    return np.float32(total / N_TOTAL)



# revision 4
# speedup vs baseline: 1.4489x; 1.4489x over previous
"""OHEM cross-entropy loss kernel for Trainium2 (8 NeuronCores, Bass/Tile).

Math (matches reference.py):
    logp   = log_softmax(seg_logit, axis=1)          # [B,C,H,W], C=19
    x_l    = logp at label (ignore 255 -> class 0)
    prob   = exp(x_l)
    thr    = max(sort(prob.flatten())[MIN_KEPT*B], 0.7)
    loss   = mean(-x_l * (prob < thr))

Device strategy (data-parallel over B across 8 cores, one image per core):
    The loss is a global mean over pixels, so any per-core pixel permutation
    is admissible. The host sorts each core's pixels by label; adjacent
    pixel PAIRS then share a label (<= 18 mismatched pairs per core, error
    ~1e-5). That unlocks a pair-packed label gather: exp values are f16, a
    pixel pair is one i32, and the 18-step class mux tree runs on i32 pairs
    via copy_predicated (which is hard-capped at 1 elem/cycle on DVE), at
    half the element count. Masks are per-pair label bit-planes.

    Per 128x512-pixel chunk:
      - one fat DMA loads [128, 19, 512] f32 logits (class-major)
      - ACT: ONE exp instruction -> eb f16 [P, C, F]
      - DVE: pairwise tree adds (f16 2x mode) -> sumexp (f16)
      - DVE: mux-tree gather on eb as [P, C, F/2] i32 pairs, masks are
        label bit-planes broadcast across class slots (5 instructions)
      - ACT: one Ln over [sumexp | e_l] packed tile -> lse, ln(e_l)
      - DVE: u = (ln(e_l) - log0.7) - lse (f16); tensor_scalar accumulate
        sum(min(u,0)) and count(u<0) into f32 partials
    The issue order is software-pipelined (chunk j's post-Ln DVE work is
    issued during chunk j+1) so the in-order ACT/DVE queues never stall
    on each other's results.

    Host combines partials: sum(-x_l*w) = -(sum_min + log(.7)*count),
    falling back to an exact host path if count <= MIN_KEPT*B (never for
    the target distribution).
"""

import numpy as np

B = 8
C = 19
H, W = 512, 1024
HW = H * W            # 524288 pixels per image/core
P = 128               # SBUF partitions
FREE = HW // P        # 4096 pixels per partition
F = 512               # chunk free size (pixels)
FP = F // 2           # pixel pairs per chunk row
NCHUNK = FREE // F    # 8
NBITS = 5             # ceil(log2(19))
C0 = float(np.log(np.float32(0.7)))
MIN_KEPT = 100000
IGNORE_INDEX = 255
N_TOTAL = B * HW

_CACHE = {}


def _build_nc():
    import bass_rust as _bass_rust
    import concourse.bacc as bacc
    import concourse.mybir as mybir
    import concourse.tile as tile
    from concourse.hw_specs import get_activation_tables

    fp32 = mybir.dt.float32
    fp16 = mybir.dt.float16
    i32 = mybir.dt.int32
    u8 = mybir.dt.uint8

    class _Bacc(bacc.Bacc):
        def insert_act_table_loads(self):
            """Same as Bacc.insert_act_table_loads, but masks Exp/Ln out of
            every act-func set except natural_log_exp_and_others (list
            positions/IDs preserved), so alternating Exp/Ln activations all
            resolve to the one set that holds both -> 1 table load instead
            of 2 per chunk (saves ~1.3us x 15 on the Scalar engine)."""
            has_activation = any(
                isinstance(i, mybir.InstActivation)
                for b in self.main_func.blocks
                for i in b.instructions
            )
            if not has_activation:
                return
            both = {
                mybir.ActivationFunctionType.Exp,
                mybir.ActivationFunctionType.Ln,
            }
            tables = [
                (name, fns if name == "natural_log_exp_and_others" else fns - both)
                for name, fns in get_activation_tables(self.m.arch).items()
            ]
            _bass_rust.insert_act_table_loads(self, tables)

    nc = _Bacc()
    logit = nc.dram_tensor("logit", [C, HW], fp32, kind="ExternalInput")
    # pair-label bit planes, chunk-major so each chunk's slice is contiguous
    bits = nc.dram_tensor(
        "bits", [P, NCHUNK, NBITS, FP], u8, kind="ExternalInput"
    )
    acc = nc.dram_tensor("acc", [P, 2 * NCHUNK], fp32, kind="ExternalOutput")

    # [C, (P FREE)] -> [P, C, FREE] view for chunked class-major loads
    logit_v = logit[:, :].rearrange("c (p f) -> p c f", p=P)

    # class mux-tree merge levels: (out_slots, data_slots, bit)
    # level 0: (2i)<-(2i+1) on bit0; level 1: (4i)<-(4i+2) on bit1; ...
    LEVELS = [
        (slice(0, 18, 2), slice(1, 19, 2), 0, 9),
        (slice(0, 17, 4), slice(2, 19, 4), 1, 5),
        (slice(0, 9, 8), slice(4, 13, 8), 2, 2),
        (slice(0, 1), slice(8, 9), 3, 1),
        (slice(0, 1), slice(16, 17), 4, 1),
    ]

    with tile.TileContext(nc) as tc:
        with (
            tc.tile_pool(name="lb", bufs=2) as lb_pool,
            tc.tile_pool(name="eb", bufs=2) as eb_pool,
            tc.tile_pool(name="sc", bufs=2) as s_pool,
            tc.tile_pool(name="bits", bufs=2) as bits_pool,
            tc.tile_pool(name="q", bufs=3) as q_pool,
            tc.tile_pool(name="pix", bufs=3) as pix_pool,
            tc.tile_pool(name="accp", bufs=1) as acc_pool,
        ):
            acc_t = acc_pool.tile([P, 2 * NCHUNK], fp32)

            prev = None  # (q, lnq) of previous chunk, for pipelined tail

            def tail(j, q, lnq):
                # u = (ln(e_l) - C0) - lse  (all f16; one DVE pass)
                u = pix_pool.tile([P, F], fp16, tag="u")
                nc.vector.scalar_tensor_tensor(
                    out=u[:], in0=lnq[:, 1, :], scalar=C0, in1=lnq[:, 0, :],
                    op0=mybir.AluOpType.subtract, op1=mybir.AluOpType.subtract,
                )
                scr = pix_pool.tile([P, F], fp16, tag="scr")
                nc.vector.tensor_scalar(
                    out=scr[:], in0=u[:], scalar1=0.0, scalar2=None,
                    op0=mybir.AluOpType.min, op1=mybir.AluOpType.add,
                    accum_out=acc_t[:, j : j + 1],
                )
                scr2 = pix_pool.tile([P, F], fp16, tag="scr2")
                nc.vector.tensor_scalar(
                    out=scr2[:], in0=u[:], scalar1=0.0, scalar2=None,
                    op0=mybir.AluOpType.is_lt, op1=mybir.AluOpType.add,
                    accum_out=acc_t[:, NCHUNK + j : NCHUNK + j + 1],
                )

            for j in range(NCHUNK):
                lb = lb_pool.tile([P, C, F], fp32, tag="lb")
                nc.sync.dma_start(out=lb[:], in_=logit_v[:, :, j * F : (j + 1) * F])
                bt = bits_pool.tile([P, NBITS, FP], u8, tag="bt")
                nc.sync.dma_start(out=bt[:], in_=bits[:, j, :, :])

                # ACT: one fat exp f32 -> f16
                eb = eb_pool.tile([P, C, F], fp16, tag="eb")
                nc.scalar.activation(
                    out=eb[:, :, :], in_=lb[:, :, :],
                    func=mybir.ActivationFunctionType.Exp,
                )
                # ACT: previous chunk's Ln now (its inputs are long ready);
                # keeps ACT queue from stalling on this chunk's tree.
                if prev is not None:
                    pj, pq, plnq = prev
                    nc.scalar.activation(
                        out=plnq[:, :, :], in_=pq[:, :, :],
                        func=mybir.ActivationFunctionType.Ln,
                    )

                # DVE sumexp tree (f16 2x), first level out-of-place to keep
                # eb intact for the gather
                s = s_pool.tile([P, 9, F], fp16, tag="s")
                nc.vector.tensor_tensor(
                    out=s[:, 0:9, :], in0=eb[:, 0:9, :], in1=eb[:, 9:18, :],
                    op=mybir.AluOpType.add,
                )
                nc.vector.tensor_tensor(
                    out=s[:, 0:4, :], in0=s[:, 0:4, :], in1=s[:, 4:8, :],
                    op=mybir.AluOpType.add,
                )
                nc.vector.tensor_tensor(
                    out=s[:, 8, :], in0=s[:, 8, :], in1=eb[:, 18, :],
                    op=mybir.AluOpType.add,
                )
                nc.vector.tensor_tensor(
                    out=s[:, 0:2, :], in0=s[:, 0:2, :], in1=s[:, 2:4, :],
                    op=mybir.AluOpType.add,
                )
                nc.vector.tensor_tensor(
                    out=s[:, 0, :], in0=s[:, 0, :], in1=s[:, 1, :],
                    op=mybir.AluOpType.add,
                )
                # q[:,0,:] = sumexp, q[:,1,:] = e_l (after merges)
                q = q_pool.tile([P, 2, F], fp16, tag="q")
                nc.vector.tensor_tensor(
                    out=q[:, 0, :], in0=s[:, 0, :], in1=s[:, 8, :],
                    op=mybir.AluOpType.add,
                )

                # label mux-tree gather on i32 pixel pairs, one instruction
                # per level with the bit-plane mask broadcast across slots
                for out_sl, data_sl, k, n in LEVELS:
                    out_ap = eb[:, out_sl, :].bitcast(i32)
                    data_ap = eb[:, data_sl, :].bitcast(i32)
                    mask_ap = bt[:, k : k + 1, :].broadcast_to((P, n, FP))
                    nc.vector.copy_predicated(
                        out=out_ap, mask=mask_ap, data=data_ap
                    )
                nc.vector.tensor_copy(out=q[:, 1, :], in_=eb[:, 0, :])

                if prev is not None:
                    tail(prev[0], prev[1], prev[2])
                lnq = q_pool.tile([P, 2, F], fp16, tag="lnq")
                prev = (j, q, lnq)

            # drain the last chunk
            pj, pq, plnq = prev
            nc.scalar.activation(
                out=plnq[:, :, :], in_=pq[:, :, :],
                func=mybir.ActivationFunctionType.Ln,
            )
            tail(pj, pq, plnq)

            nc.sync.dma_start(out=acc[:, :], in_=acc_t[:])
    nc.finalize()
    return nc


def _host_fallback(seg_logit, seg_label):
    """Exact numpy replication of the reference (quantile path included)."""
    x = np.asarray(seg_logit, dtype=np.float32)
    lbl = np.asarray(seg_label)
    Bn, Cn = x.shape[0], x.shape[1]
    xf = x.reshape(Bn, Cn, -1)
    m = xf.max(axis=1, keepdims=True)
    e = np.exp(xf - m)
    lse = np.log(e.sum(axis=1, keepdims=True)) + m
    logp = xf - lse
    l2 = np.where(lbl == IGNORE_INDEX, 0, lbl).reshape(Bn, 1, -1).astype(np.int64)
    lp_at = np.take_along_axis(logp, l2, axis=1)[:, 0]
    prob = np.exp(lp_at)
    sortp = np.sort(prob.reshape(-1))
    idx = min(MIN_KEPT * Bn, sortp.shape[0] - 1)
    thr = max(float(sortp[idx]), np.float32(0.7))
    wgt = (prob < thr).astype(np.float32)
    return np.float32((-lp_at * wgt).mean())


def kernel(seg_logit, seg_label):
    from concourse import bass_utils

    x = np.ascontiguousarray(np.asarray(seg_logit, dtype=np.float32)).reshape(
        B, C, HW
    )
    lbl = np.asarray(seg_label)
    lbl = np.where(lbl == IGNORE_INDEX, 0, lbl).astype(np.uint8).reshape(B, HW)

    in_maps = []
    for b in range(B):
        order = np.argsort(lbl[b], kind="stable")
        xs = np.ascontiguousarray(x[b][:, order])
        ls = lbl[b][order]
        # pair label = label of the even element of each pair
        plr = ls[0::2].reshape(P, NCHUNK, FP)          # [P, NCHUNK, FP]
        bitsp = np.stack(
            [((plr >> k) & 1).astype(np.uint8) for k in range(NBITS)], axis=2
        )                                               # [P, NCHUNK, NBITS, FP]
        in_maps.append({"logit": xs, "bits": np.ascontiguousarray(bitsp)})

    if "nc" not in _CACHE:
        _CACHE["nc"] = _build_nc()
    nc = _CACHE["nc"]

    res = bass_utils.run_bass_kernel_spmd(nc, in_maps, core_ids=list(range(B)))

    racc = 0.0
    wacc = 0.0
    for r in res.results:
        a = r["acc"]
        racc += float(a[:, :NCHUNK].sum(dtype=np.float64))
        wacc += float(a[:, NCHUNK:].sum(dtype=np.float64))

    if wacc <= MIN_KEPT * B:
        # quantile threshold exceeds 0.7 -> exact host path (rare/never for
        # the target distribution)
        return _host_fallback(seg_logit, seg_label)

    total = -(racc + C0 * wacc)
    return np.float32(total / N_TOTAL)
